# revision 1
# baseline (speedup 1.0000x reference)
"""Trainium2 Bass kernel for nn_AttentionHead_51178830299302.

Single attention head: B=8, S=2048, E=1024, H=64, fp32 I/O, decoder
(causal) masking plus a pad-pad coupling term (padded queries attend
bidirectionally to padded keys).

Strategy:
  * Data-parallel over batch: one batch element per NeuronCore (8 cores).
  * Host-side, each sequence is stably partitioned into [pad | valid]
    positions (order preserved within each group).  The masked softmax
    then decomposes exactly into two independent attention problems:
      - pad x pad with full bidirectional softmax (no mask),
      - valid x valid with plain causal masking,
    which skips ~60% of the S x S exp/matmul work vs. the dense mask.
    Pad goes FIRST so its (scalar-exp-heavy, fp8 DoubleRow) attention
    becomes eligible early and overlaps the remaining projections; the
    causal part's PE-light trapezoid blocks form the tail.
  * bf16 matmul pipeline (fp32 PSUM accumulation), exp on ScalarE
    straight from PSUM with a -3 bias folded in (cancels in the final
    divide; keeps exp within fp8 e4m3 range).  The pad part's softmax
    weights and V are fp8 e4m3: its AV contracts 2 key-chunks per
    matmul via DoubleRow (~1.5x), and quantization washes out over
    ~1000-key averages (the causal part stays bf16 because its early
    rows average few keys).
  * All matmuls are wide streams: q/k/v projections produce transposed
    [head, seq] layouts; V is moved to its natural layout with one XBAR
    transpose DMA; attention output is produced transposed [H+1, seq]
    with the softmax row-sum riding along as an appended ones-row of V;
    the final divide+transpose happens on host.
  * Slot-padded keys are killed via one augmented contraction row
    (score += kill_j * NEG); causal masking inside diagonal 128-blocks
    is one bf16 multiply with a constant 0/1 tril strip, and all work
    on a diagonal block is restricted to its unmasked trapezoid.
  * DMA: HWDGE transfers are FIFO per trigger ring (Scalar + Sync), so
    bulk hidden-state half-slices alternate rings in consumption order
    while small latency-critical transfers (weights, bounces, V
    transposes, outputs) slot between them; tiny constants ride the
    slow GpSimd SWDGE.  PE warm-up matmuls ramp the p-state while the
    first slice streams in.

kernel(**inputs) takes the FULL unsharded fp32 inputs and returns the
FULL [8, 2048, 64] fp32 output.
"""

import numpy as np
import ml_dtypes

B, S, E, H = 8, 2048, 1024, 64
NEG = -100000.0
P = 128
BF = ml_dtypes.bfloat16
F8 = ml_dtypes.float8_e4m3

_NC_CACHE: dict = {}


def _patch_tile_drain():
    """The stock TileContext exit hangs every global-clock wait on a single
    Drain instruction; this container's walrus caps sync waits at 1 per
    instruction.  Split the waits across single-wait nops, and drop the
    second (post-semclear) all-engine barrier — engines halt right after,
    and NEFF re-execution only starts once every engine has halted."""
    import concourse.tile as tile
    import concourse.mybir as mybir
    from bass_rust import ScopedClock

    if getattr(tile.TileContext, "_drain_waits_split", False):
        return

    def _drain_and_barrier(self, tick_clock, wait_clock):
        nc = self.nc
        carrier = nc.sync.nop(nofuse=True)
        wait_clock.add_sem_waits(
            carrier.ins, ScopedClock({None: tick_clock.global_clock})
        )
        si = carrier.ins.sync_info
        waits = list(si.on_wait) if si and si.on_wait else []
        if len(waits) > 1:
            si.on_wait = waits[:1]
            for w in waits[1:]:
                n = nc.sync.nop(nofuse=True)
                nsi = n.ins.sync_info
                if nsi is None:
                    n.ins.sync_info = mybir.SyncInfo(on_wait=[w], on_update=[])
                else:
                    nsi.on_wait = [w]
        nc.sync.drain()
        nc.all_engine_barrier(sem_only=True)
        popped = nc._tile_sem_poison_stack.pop()
        assert popped is self._sem_poison
        nc.clear_and_free_semaphores(list(self.sems.allocated().values()))

    tile.TileContext._drain_and_barrier = _drain_and_barrier
    tile.TileContext._drain_waits_split = True


def _patch_sync_wait_split():
    """This container's walrus codegen rejects instructions carrying more
    than one sync wait.  Post-process the serialized BIR: hoist excess
    waits onto injected NoOps on the same engine, just before the
    instruction (the sequencer executes them in order, so semantics are
    preserved)."""
    import json
    import concourse.bass as bass

    if getattr(bass.Bass, "_sync_wait_split", False):
        return
    orig = bass.Bass.to_json_bytes

    def to_json_bytes(self) -> bytes:
        j = json.loads(orig(self))
        ctr = [0]

        def fix_block(blk):
            insts = blk.get("instructions")
            if not isinstance(insts, list):
                return
            out = []
            for inst in insts:
                si = inst.get("sync_info")
                ow = (si or {}).get("on_wait") or []
                if len(ow) > 1:
                    si["on_wait"] = ow[-1:]
                    for w in ow[:-1]:
                        ctr[0] += 1
                        out.append(
                            {
                                "debug": inst.get("debug", 0),
                                "engine": inst["engine"],
                                "ins": [],
                                "name": f"I-wsplit-{ctr[0]}",
                                "opcode": "NoOp",
                                "outs": [],
                                "sync_info": {"on_wait": [w], "on_update": []},
                            }
                        )
                out.append(inst)
            blk["instructions"] = out

        def rec(o):
            if isinstance(o, dict):
                if "instructions" in o:
                    fix_block(o)
                for v in o.values():
                    rec(v)
            elif isinstance(o, list):
                for v in o:
                    rec(v)

        rec(j)
        return json.dumps(j).encode()

    bass.Bass.to_json_bytes = to_json_bytes
    bass.Bass._sync_wait_split = True


def build_nc(SV: int, SP: int):
    """Build the SPMD per-core Bass program.

    Per-core DRAM tensors:
      hsT  [E, SVP]   bf16   sorted hidden state, transposed (E-major)
      wqk  [E, 128]   bf16   [Wq/sqrt(H) | Wk]
      wv   [E, H]     bf16
      bqk  [128, 1]   f32    [bq/sqrt(H) ; bk]
      kill [1, SVP]   bf16   1.0 on slot-padding positions
      c01  [128,1024] bf16   tril keep-mask: c01[j, 512+y] = (j <= y)
      outT [65, SVP]  f32    rows 0..63 unnormalized output^T, row 64
                             softmax denominators (host divides)
    """
    import concourse.bass as bass
    import concourse.mybir as mybir
    import concourse.tile as tile
    from contextlib import ExitStack

    _patch_tile_drain()
    _patch_sync_wait_split()
    bf, f32, f16 = mybir.dt.bfloat16, mybir.dt.float32, mybir.dt.float16
    f8 = mybir.dt.float8e4
    DR = mybir.MatmulPerfMode.DoubleRow
    Exp = mybir.ActivationFunctionType.Exp

    SVP = SV + SP
    NKC_V, NKC_P = SV // P, SP // P
    NT = SVP // P

    nc = bass.Bass("TRN2", target_bir_lowering=False, debug=False)
    NSL = (SVP + 511) // 512  # 512-col projection slices
    # hsT packed slice-major: [128, NSL, 8, 512]; per partition each slice
    # is one contiguous 8 KiB run -> 128 maximal DMA descriptors per slice.
    hsT_d = nc.dram_tensor("hsT", [P, NSL, 8, 512], bf, kind="ExternalInput").ap()
    wqk_d = nc.dram_tensor("wqk", [P, 8, P], bf, kind="ExternalInput").ap()
    wv_d = nc.dram_tensor("wv", [P, 8, H], bf, kind="ExternalInput").ap()
    bqk_d = nc.dram_tensor("bqk", [P, 1], f32, kind="ExternalInput").ap()
    kill_d = nc.dram_tensor("kill", [2, SVP], bf, kind="ExternalInput").ap()
    c01_d = nc.dram_tensor("c01", [P, 1024], bf, kind="ExternalInput").ap()
    outT_d = nc.dram_tensor("outT", [H + 1, SVP], f32, kind="ExternalOutput").ap()

    with tile.TileContext(nc) as tc, ExitStack() as ctx:
        singles = ctx.enter_context(tc.tile_pool(name="singles", bufs=1))

        # PE warm-up source tile.  GpSimd is the first engine out of the
        # NEFF preamble (~0.7us before Vector/Scalar/Sync), so the
        # memsets and the first bulk DMA triggers go there.
        wz = singles.tile([P, 512], bf)
        nc.gpsimd.memset(wz[:], 0.0)
        ones_t = singles.tile([1, P], bf)
        nc.gpsimd.memset(ones_t[:], 1.0)

        wqk_s = singles.tile([P, 8, P], bf)
        wv_s = singles.tile([P, 8, H], bf)
        bqk_s = singles.tile([P, 1], f32)
        c01_s = singles.tile([P, 1024], bf)
        exp_warm = singles.tile([1, 1], bf)
        # exp runs as exp(s - 3) so the pad path stays inside fp8 e4m3
        # range; the factor e^-3 cancels in the host num/den divide.
        nbias = singles.tile([P, 1], f32)
        nc.gpsimd.memset(nbias[:], -3.0)

        # qT/kT: 64 head rows + 1 augmented mask row (row 64).
        # score += qT_aug[64] * kT_aug[64] = NEG * kill_j
        qT = singles.tile([65, SVP], bf)
        kT = singles.tile([65, SVP], bf)

        # V in natural [seq-part, head] layout with an appended ones
        # column (row-sums of the attention weights ride along in the
        # AV matmul as output row H).  Layout is PAD-FIRST: the pad part
        # (bidirectional, scalar-exp-heavy, fp8 DoubleRow AV) becomes
        # eligible as soon as slices 0..2 are projected and overlaps the
        # remaining projections; the causal valid part (PE-light
        # trapezoids, bf16 AV for small-neff early rows) forms the tail.
        # The two parts contract disjoint key chunks, so V splits:
        # fp8 for pad chunks 0..NKC_P-1 (free dim padded 65->80 for the
        # 16B DoubleRow chunk stride), bf16 for valid chunks.
        vS8 = singles.tile([P, NKC_P, 80], f8)
        nc.gpsimd.memset(vS8[:, :, H : H + 1], 1.0)
        vS_bf = singles.tile([P, NKC_V, H + 1], bf)
        nc.gpsimd.memset(vS_bf[:, :, H : H + 1], 1.0)
        vT = singles.tile([P, SVP], bf)
        # XBAR transpose needs a contiguous destination on HW; stage here,
        # then strided-copy into vS (which carries the ones column).
        vN = singles.tile([P, NT, H], bf)

        # hidden state, E-major, packed slice-major so each slice is one
        # 1MB transfer of maximal descriptors.  Bulk loads go on the two
        # HWDGE trigger engines (Scalar/Sync) -- GpSimd's software DGE
        # runs at ~1/5 the bandwidth, so it only carries the small
        # latency-tolerant constants.  Transfers land on round-robined
        # HW DMA queues and run in parallel; only the ~0.65us trigger
        # busy-time serializes per engine.
        hsT = singles.tile([P, NSL, 8, 512], bf)
        hsT_loaded = [False] * NSL

        def load_hsT(si):
            # Half-slice transfers: Tile's subtile deps release each
            # projection matmul as soon as its half lands.  The two
            # halves of EACH slice go to different rings (scalar + sync)
            # so they transfer concurrently and the slice completes in
            # half the time -- slice completion order, not just byte
            # delivery, is what gates the projection stream.
            if si < NSL and not hsT_loaded[si]:
                hsT_loaded[si] = True
                for _h, eng in ((0, nc.scalar), (1, nc.sync)):
                    eng.dma_start(
                        hsT[:, si, 4 * _h : 4 * _h + 4, :],
                        hsT_d[:, si, 4 * _h : 4 * _h + 4, :],
                    )

        # Ring plan: HWDGE DMAs are FIFO per issuing engine, so ring
        # ORDER is the prioritization tool.  Slices 0..2 fire up-front;
        # 3,4 are prefetched inside the emission loop so the k-bounce /
        # v-transpose / output DMAs aren't queued behind them.  Tiny
        # constants ride the slow GpSimd SWDGE.  A dummy exp on Scalar
        # pulls the 1.3us ACT_TABLE_LOAD out of the attention pipeline.
        # wqk leads the scalar ring (first projection matmul needs it
        # and chunk-0 alone unblocks the stream); slice 1 leads the sync
        # ring so qk1 follows qk0 without a ring-order stall.
        nc.scalar.dma_start(wqk_s[:, 0:1, :], wqk_d[:, 0:1, :])
        nc.scalar.dma_start(wqk_s[:, 1:8, :], wqk_d[:, 1:8, :])
        load_hsT(0)
        load_hsT(1)
        nc.scalar.activation(exp_warm[:], ones_t[0:1, 0:1], Exp)
        nc.scalar.dma_start(wv_s[:], wv_d)
        nc.scalar.dma_start(c01_s[:], c01_d)
        load_hsT(2)
        nc.gpsimd.dma_start(bqk_s[:], bqk_d)
        nc.gpsimd.dma_start(kT[64:65, :], kill_d[0:1, :])
        nc.gpsimd.dma_start(qT[64:65, :], kill_d[1:2, :])

        # ------- interleaved projections + attention -------
        # Emission order interleaves projection slices with attention
        # q-blocks whose inputs are already covered, so the PE stream has
        # no phase barrier and HAM stays warm.
        with tc.tile_pool(name="pp", bufs=2, space="PSUM") as pp, \
             tc.tile_pool(name="acc", bufs=2, space="PSUM") as acc, \
             tc.tile_pool(name="spsum", bufs=4, space="PSUM") as spsum, \
             tc.tile_pool(name="kstage", bufs=2) as kstage_pool, \
             tc.tile_pool(name="wpool", bufs=4) as wpool, \
             tc.tile_pool(name="opool", bufs=2) as opool:

            # PE warm-up: fills the ~3us until hsT slice 0 lands and
            # ramps the PE p-state (0.65 -> 1.2 -> 2.4 GHz after ~3us of
            # continuous execution).  Rides the pp pool (in-order PE
            # makes the WAW reuse free).
            warm_ps = pp.tile([P, 512], f32, tag="ps", name="warm_ps")
            for _ in range(12):
                nc.tensor.matmul(
                    warm_ps[:, 0:256], lhsT=wz[:, 0:P], rhs=wz[:, 0:256],
                    start=True, stop=True,
                )

            def emit_qk_slice(sb):
                si = sb // 512
                w = min(512, SVP - sb)
                ps = pp.tile([P, 512], f32, tag="ps", name="ps")
                for c in range(8):
                    nc.tensor.matmul(
                        ps[:, :w],
                        lhsT=wqk_s[:, c, :],
                        rhs=hsT[:, si, c, :w],
                        start=(c == 0),
                        stop=(c == 7),
                    )
                # q lands on partitions 0:64 -> evacuate straight into qT
                nc.vector.tensor_scalar_add(
                    qT[0:64, sb : sb + w], ps[0:64, :w], bqk_s[0:64, 0:1]
                )
                # k lands on partitions 64:128; engines cannot shift
                # partitions, so stage and bounce via DMA to rows 0:64.
                kst = kstage_pool.tile([P, 512], bf)
                nc.vector.tensor_scalar_add(
                    kst[64:128, :w], ps[64:128, :w], bqk_s[64:128, 0:1]
                )
                nc.sync.dma_start(kT[0:64, sb : sb + w], kst[64:128, :w])

            def copy_vS(ta, tb):
                # route chunks to the part-specific V tile (pad-first:
                # fp8 then valid bf16), splitting when a slice straddles
                # SP.  On GpSimd: it is idle here and this keeps the
                # (busy) vector queue out of the AV dependency chain.
                if ta < NKC_P:
                    e = min(tb, NKC_P)
                    nc.gpsimd.tensor_copy(vS8[:, ta:e, 0:H], vN[:, ta:e, :])
                if tb > NKC_P:
                    b = max(ta, NKC_P)
                    nc.gpsimd.tensor_copy(
                        vS_bf[:, b - NKC_P : tb - NKC_P, 0:H], vN[:, b:tb, :]
                    )

            def _v_finish(s, rows, pvd):
                w = min(512, SVP - s)
                cp = nc.vector.tensor_copy(vT[rows[0] : rows[1], s : s + w],
                                           pvd[rows[0] : rows[1], :w])
                ta, tb = s // P, (s + w) // P
                nc.sync.dma_start_transpose(
                    vN[:, ta:tb, :], vT[rows[0] : rows[1], s : s + w]
                )
                copy_vS(ta, tb)
                return cp

            def emit_v_pair(sA, sB):
                # V projection for two 512-slices concurrently via PE
                # column tiling: slice A in array cols 0:64 -> PSUM rows
                # 0:64, slice B in cols 64:128 -> PSUM rows 64:128.  A
                # K=1 zero matmul opens the accumulation group for the
                # whole bank (per-chain start=True would clear the
                # sibling chain's has_written bits).
                pvd = acc.tile([P, 512], f32, tag="acc", name="pvd")
                wA = min(512, SVP - sA)
                if sB is None:
                    for c in range(8):
                        nc.tensor.matmul(
                            pvd[0:H, :wA],
                            lhsT=wv_s[:, c, :],
                            rhs=hsT[:, sA // 512, c, :wA],
                            start=(c == 0),
                            stop=(c == 7),
                        )
                    _v_finish(sA, (0, H), pvd)
                    return
                wB = min(512, SVP - sB)
                nc.tensor.matmul(
                    pvd[:, 0:512], lhsT=ones_t[:], rhs=wz[0:1, 0:512],
                    start=True, stop=True, skip_group_check=True,
                )
                for c in range(8):
                    nc.tensor.matmul(
                        pvd[0:H, :wA],
                        lhsT=wv_s[:, c, :],
                        rhs=hsT[:, sA // 512, c, :wA],
                        start=False, stop=(c == 7), tile_position=(0, 0),
                        skip_group_check=True,
                    )
                    nc.tensor.matmul(
                        pvd[H:P, :wB],
                        lhsT=wv_s[:, c, :],
                        rhs=hsT[:, sB // 512, c, :wB],
                        start=False, stop=(c == 7), tile_position=(0, H),
                        skip_group_check=True,
                    )
                # the A-half evac copies BOTH partition halves into vT
                # (rows 64:128 of cols sA are dead space) so the read
                # carries a RAW dependency on the group-closing B matmul
                # and can't be scheduled while the group is open.
                nc.vector.tensor_copy(vT[:, sA : sA + wA], pvd[:, :wA])
                ta, tb = sA // P, (sA + wA) // P
                nc.sync.dma_start_transpose(
                    vN[:, ta:tb, :], vT[0:H, sA : sA + wA]
                )
                copy_vS(ta, tb)
                _v_finish(sB, (H, P), pvd)

            def emit_qblock(part, q0r):
                # part 0 = pad (cols 0..SP, bidirectional, fp8 DR AV),
                # part 1 = valid (cols SP.., causal, bf16 AV)
                causal = part == 1
                part_q0 = 0 if part == 0 else SP
                part_len = SP if part == 0 else SV
                kc_base = 0 if part == 0 else NKC_P
                w = min(512, part_len - q0r)
                q0 = part_q0 + q0r
                if causal:
                    kcs = list(range(0, (q0r + w - 1) // P + 1))
                else:
                    kcs = list(range(NKC_P))

                ot = acc.tile([H + 1, 512], f32, tag="acc", name="ot")
                spb = 512 // w  # score slots per 1-bank PSUM group
                banks = [kcs[i : i + spb] for i in range(0, len(kcs), spb)]
                n_kc = len(kcs)
                ki = 0
                wt = None
                pend = []  # (kcr, wt_tile, slot) exp'd, awaiting AV

                def flush_av(final):
                    # The PE executes in emission order, so an AV emitted
                    # right behind its exp stalls the scores queued after
                    # it.  Keep ~2 exp'd banks pending before flushing,
                    # so the exp pipeline stays ahead of the AV consumer.
                    nonlocal ki
                    while pend:
                        if (
                            not causal
                            and len(pend) >= 2
                            and (final or len(pend) >= 4)
                            and pend[0][1] is pend[1][1]
                        ):
                            # fp8 DoubleRow: one matmul contracts both
                            # key-chunks of the pair (wt slots j, j+1)
                            (kc0, wtt, j0, _), (kc1, _, j1, _) = pend[0], pend[1]
                            assert kc1 == kc0 + 1 and j1 == j0 + 1
                            nc.tensor.matmul(
                                ot[:, :w],
                                lhsT=vS8[:, kc0 : kc0 + 2, 0 : H + 1],
                                rhs=wtt[:, j0 : j0 + 2, :w],
                                start=(ki == 0),
                                stop=(ki + 2 == n_kc),
                                perf_mode=DR,
                            )
                            ki += 2
                            del pend[:2]
                        elif causal and (final or len(pend) >= 3):
                            kc0, wtt, j0, d0 = pend[0]
                            nc.tensor.matmul(
                                ot[:, d0:w],
                                lhsT=vS_bf[:, kc0, :],
                                rhs=wtt[:, j0, d0:w],
                                start=(ki == 0),
                                stop=(ki + 1 == n_kc),
                            )
                            ki += 1
                            del pend[:1]
                        elif not causal and final:
                            kc0, wtt, j0, d0 = pend[0]
                            nc.tensor.matmul(
                                ot[:, d0:w],
                                lhsT=vS8[:, kc0, 0 : H + 1],
                                rhs=wtt[:, j0, d0:w],
                                start=(ki == 0),
                                stop=(ki + 1 == n_kc),
                            )
                            ki += 1
                            del pend[:1]
                        else:
                            break

                for bi, bank in enumerate(banks):
                    nb = len(bank)
                    if not causal:
                        if bi % 2 == 0:
                            # one wt tile spans two score banks so
                            # DoubleRow pairs always fall inside one tile
                            wt = wpool.tile(
                                [P, 2 * spb, w], f8, tag="wt", name="wt"
                            )
                        base = (bi % 2) * spb
                    else:
                        wt = wpool.tile([P, spb, w], bf, tag="wtb", name="wtb")
                        base = 0
                    st_ps = spsum.tile([P, spb, w], f32, tag="st", name="st_ps")
                    # diagonal-band slots only touch queries >= their key
                    # base: restrict scores/exp/mask/AV to cols [d0, w)
                    ds = [
                        max(kcr * P - q0r, 0) if causal else 0
                        for kcr in bank
                    ]
                    for s, kcr in enumerate(bank):
                        kc = kc_base + kcr
                        nc.tensor.matmul(
                            st_ps[:, s, ds[s] : w],
                            lhsT=kT[0:65, kc * P : (kc + 1) * P],
                            rhs=qT[0:65, q0 + ds[s] : q0 + w],
                            start=True,
                            stop=True,
                        )
                    if nb == 1:
                        nc.scalar.activation(
                            wt[:, base, ds[0] : w], st_ps[:, 0, ds[0] : w],
                            Exp, bias=nbias[:, 0:1],
                        )
                    else:
                        assert all(d == 0 for d in ds)
                        nc.scalar.activation(
                            wt[:, base : base + nb, :], st_ps[:, 0:nb, :], Exp,
                            bias=nbias[:, 0:1],
                        )
                    if causal:
                        for s, kcr in enumerate(bank):
                            if kcr * P - q0r >= 0:  # diagonal-band block:
                                # only the 128-wide strip at d needs the
                                # intra-block tril; later cols are all-keep
                                d0 = ds[s]
                                de = min(d0 + P, w)
                                nc.vector.tensor_mul(
                                    wt[:, base + s, d0:de],
                                    wt[:, base + s, d0:de],
                                    c01_s[:, 512 : 512 + de - d0],
                                )
                    for s, kcr in enumerate(bank):
                        pend.append((kcr, wt, base + s, ds[s]))
                    flush_av(final=(bi == len(banks) - 1))

                osb = opool.tile([H + 1, 512], f32)
                nc.vector.tensor_copy(osb[:, :w], ot[:, :w])
                nc.sync.dma_start(outT_d[:, q0 : q0 + w], osb[:, :w])

            # schedule: proj slice i covers seq cols [512i, 512i+512);
            # a q-block may be emitted once the slices covering both its
            # queries and its keys (and V tiles) have been emitted.
            n_slices = (SVP + 511) // 512
            qblocks = []  # (part, q0r, need_cols)
            for part in range(2):
                part_q0 = 0 if part == 0 else SP
                part_len = SP if part == 0 else SV
                for q0r in range(0, part_len, 512):
                    w = min(512, part_len - q0r)
                    if part == 1:  # causal: keys up to the diagonal
                        kmax = SP + ((q0r + w - 1) // P + 1) * P
                    else:  # pad: all pad keys (cols 0..SP)
                        kmax = SP
                    need = max(part_q0 + q0r + w, kmax)
                    qblocks.append((part, q0r, need))
            qi = 0
            slice_starts = list(range(0, SVP, 512))
            for pi in range(0, len(slice_starts), 2):
                sA = slice_starts[pi]
                sB = slice_starts[pi + 1] if pi + 1 < len(slice_starts) else None
                if sB is not None and min(512, SVP - sB) != min(512, SVP - sA):
                    sB = None  # unequal widths: emit separately below

                load_hsT(pi)
                load_hsT(pi + 1)
                sB_real = slice_starts[pi + 1] if pi + 1 < len(slice_starts) else None
                if sB_real is None:
                    # lone tail slice: V first -- the pad AV chain waits
                    # on the V transpose pipeline, qk evac is cheaper
                    emit_v_pair(sA, None)
                    emit_qk_slice(sA)
                else:
                    emit_qk_slice(sA)
                    emit_qk_slice(sB_real)
                    emit_v_pair(sA, sB)
                    if sB is None:
                        emit_v_pair(sB_real, None)
                # prefetch the next pair only after this pair's bounce /
                # transpose / output DMAs are queued, so the bulk loads
                # don't delay them in the ring FIFO
                load_hsT(pi + 2)
                load_hsT(pi + 3)
                covered = min((sB_real if sB_real is not None else sA) + 512, SVP)
                while qi < len(qblocks) and qblocks[qi][2] <= covered:
                    emit_qblock(qblocks[qi][0], qblocks[qi][1])
                    qi += 1
            while qi < len(qblocks):
                emit_qblock(qblocks[qi][0], qblocks[qi][1])
                qi += 1
    return nc


def _prepare(hidden_state, attention_masks, Wq, bq, Wk, bk, Wv, bv):
    """Host-side shard prep: sort each sequence into [valid | pad],
    pad both groups to shared multiples of 128, cast to bf16."""
    hs = np.asarray(hidden_state, dtype=np.float32)
    m = np.asarray(attention_masks)
    perms, nvs = [], []
    for b in range(B):
        mb = np.asarray(m[b]).astype(np.int64)
        perms.append(np.argsort(1 - mb, kind="stable"))
        nvs.append(int(mb.sum()))
    nps = [S - nv for nv in nvs]
    SV = max(128, -(-max(nvs) // P) * P)
    SPn = max(128, -(-max(nps) // P) * P)
    SVP = SV + SPn

    wqk = np.ascontiguousarray(
        np.concatenate(
            [np.asarray(Wq, np.float32) / np.sqrt(H), np.asarray(Wk, np.float32)],
            axis=1,
        ).reshape(8, P, P).transpose(1, 0, 2)
    ).astype(BF)  # [p, c, m]
    wv = np.ascontiguousarray(
        np.asarray(Wv, np.float32).reshape(8, P, H).transpose(1, 0, 2)
    ).astype(BF)  # [p, c, m]
    bqk = np.concatenate(
        [np.asarray(bq, np.float32) / np.sqrt(H), np.asarray(bk, np.float32)]
    ).reshape(P, 1).astype(np.float32)

    # c01[j, 512+y] = 1.0 iff j <= y   (keep when q_rel - d >= j)
    y = np.arange(1024) - 512
    c01 = (np.arange(P)[:, None] <= y[None, :]).astype(BF)

    in_maps = []
    for b in range(B):
        nv, npd = nvs[b], nps[b]
        NSL = (SVP + 511) // 512
        # pad-first layout: pad rows at cols [0, npd), valid rows at
        # [SPn, SPn + nv); slot-padding in between is killed
        hs_sorted = np.zeros((NSL * 512, E), np.float32)
        hs_sorted[:npd] = hs[b][perms[b][nv:]]
        hs_sorted[SPn : SPn + nv] = hs[b][perms[b][:nv]]
        # pack [128, NSL, 8, 512]: hsT[p, si, c, j] = hs_sorted[si*512+j, c*128+p]
        hsT = np.ascontiguousarray(
            hs_sorted.reshape(NSL, 512, 8, P).transpose(3, 0, 2, 1)
        ).astype(BF)
        kill = np.zeros((2, SVP), np.float32)
        kill[0, npd:SPn] = 1.0
        kill[0, SPn + nv :] = 1.0
        kill[1, :] = NEG
        in_maps.append(
            {
                "hsT": hsT,
                "wqk": wqk,
                "wv": wv,
                "bqk": bqk,
                "kill": kill.astype(BF),
                "c01": c01,
            }
        )
    return in_maps, perms, nvs, SV, SPn


def _run(inputs: dict, trace: bool = False):
    from concourse import bass_utils

    in_maps, perms, nvs, SV, SPn = _prepare(**inputs)
    key = (SV, SPn)
    if key not in _NC_CACHE:
        _NC_CACHE[key] = build_nc(SV, SPn)
    nc = _NC_CACHE[key]

    res = bass_utils.run_bass_kernel_spmd(
        nc, in_maps, core_ids=list(range(8)), trace=trace
    )

    bv = np.asarray(inputs["bv"], np.float32)
    out = np.empty((B, S, H), np.float32)
    for b in range(B):
        ot = np.asarray(res.results[b]["outT"], np.float32)  # [65, SVP]
        with np.errstate(divide="ignore", invalid="ignore", over="ignore"):
            dev = (ot[:H] / ot[H]).T  # normalized; slot-pad rows are discarded
        nv = nvs[b]
        out[b][perms[b][nv:]] = dev[: S - nv]
        out[b][perms[b][:nv]] = dev[SPn : SPn + nv]
    out += bv  # v-projection bias commutes with the softmax average
    return out, res


def kernel(**inputs) -> np.ndarray:
    out, _ = _run(inputs, trace=False)
    return out

